# revision 21
# baseline (speedup 1.0000x reference)
# Multi-head attention (B=2, S=2048, D=1024, H=16) on 8 TRN2 NeuronCores.
#
# Sharding: core c handles batch b = c//4 and head-group hg = c%4 (4 heads,
# channel slice J = hg*256 : (hg+1)*256).  Each core computes
#   Q^T/K^T = W^T x^T (+bias), V = x W (+bias),
#   S^T_h = K_h^T^T-contraction (d on partitions)  -> exp on ScalarE,
#   O^T_h = [V_h | 1]^T P^T_h  (row 64 = softmax denominator),
#   y_partial = O^T^T Wo_slice    (bf16, [S, D])
# Host sums the 4 partials per batch and adds bo.
#
# Layout notes:
#  - Contraction dim always on SBUF partitions: x^T, W natural, Q^T/K^T with
#    head-dim on partitions, P^T with key-dim on partitions.
#  - Softmax denominators are handled with the fused ones-column trick; the
#    per-q reciprocal is broadcast across partitions via gpsimd
#    partition_broadcast (DVE lanes cannot cross partitions).
#  - Head pairs are stacked on partitions (64+64) so S^T matmuls of the two
#    heads run concurrently in disjoint PE row-groups (K=64 tile packing).

import numpy as np

B = 2
S = 2048
D = 1024
H = 16
DH = 64
NCORES = 8
HL = 4            # heads per core
J = HL * DH       # 256: per-core channel slice of D
PAIRS = 2         # head-pairs per core

_cache = {}
USE_APPROX_RECIP = True


def _build_module(seq=S):
    import concourse.bass as bass
    import concourse.mybir as mybir
    import concourse.tile as tile

    from concourse import bacc

    dt = mybir.dt
    f32 = dt.float32
    bf16 = dt.bfloat16
    AF = mybir.ActivationFunctionType

    KB = seq // 128          # key blocks (partition tiles of the key dim)
    QC = min(512, seq)       # query chunk (matmul free dim)
    NQ = seq // QC           # query chunks
    NCH = min(512, seq)      # projection free-dim chunk
    NP = seq // NCH          # projection chunks
    KT = D // 128            # contraction tiles for projections (8)
    G2 = KB // 2             # pairs of key-blocks per psum-S tile

    nc = bacc.Bacc("TRN2", target_bir_lowering=False, debug=False)

    xT_d = nc.dram_tensor("xT", [D, seq], bf16, kind="ExternalInput").ap()
    wq_d = nc.dram_tensor("wq", [D, J], bf16, kind="ExternalInput").ap()
    wk_d = nc.dram_tensor("wk", [D, J], bf16, kind="ExternalInput").ap()
    wv_d = nc.dram_tensor("wv", [D, J], bf16, kind="ExternalInput").ap()
    wo_d = nc.dram_tensor("wo", [J, D], bf16, kind="ExternalInput").ap()
    bqk_d = nc.dram_tensor("bqk", [1, 512], bf16, kind="ExternalInput").ap()
    bv_d = nc.dram_tensor("bv", [1, J], bf16, kind="ExternalInput").ap()
    y_d = nc.dram_tensor("y", [seq, D], bf16, kind="ExternalOutput").ap()

    with tile.TileContext(nc) as tc:
        import contextlib
        ctx = contextlib.ExitStack()
        with ctx:
            xt_pool = ctx.enter_context(tc.tile_pool(name="xt", bufs=1))
            w_pool = ctx.enter_context(tc.tile_pool(name="w", bufs=1))
            qk_pool = ctx.enter_context(tc.tile_pool(name="qk", bufs=1))
            v_pool = ctx.enter_context(tc.tile_pool(name="v", bufs=1))
            pt_pool = ctx.enter_context(tc.tile_pool(name="pt", bufs=2))
            ot_pool = ctx.enter_context(tc.tile_pool(name="ot", bufs=1))
            sm_pool = ctx.enter_context(tc.tile_pool(name="sm", bufs=2))
            yb_pool = ctx.enter_context(tc.tile_pool(name="yb", bufs=2))
            psS_pool = ctx.enter_context(
                tc.tile_pool(name="psS", bufs=2, space="PSUM"))
            psO_pool = ctx.enter_context(
                tc.tile_pool(name="psO", bufs=2, space="PSUM"))
            mm_pool = ctx.enter_context(
                tc.tile_pool(name="mm", bufs=2, space="PSUM"))
            dram_pool = ctx.enter_context(
                tc.tile_pool(name="dscr", bufs=2, space="DRAM"))

            # ---- persistent SBUF tensors + input DMAs ----
            xt_sb = [xt_pool.tile([128, seq], bf16, tag=f"xt{k}",
                                  name=f"xt{k}") for k in range(KT)]
            wq_sb = [w_pool.tile([128, J], bf16, tag=f"wq{k}",
                                 name=f"wq{k}") for k in range(KT)]
            wk_sb = [w_pool.tile([128, J], bf16, tag=f"wk{k}",
                                 name=f"wk{k}") for k in range(KT)]
            wv_sb = [w_pool.tile([128, J], bf16, tag=f"wv{k}",
                                 name=f"wv{k}") for k in range(KT)]
            wo_sb = [w_pool.tile([128, D], bf16, tag=f"wo{p}",
                                 name=f"wo{p}") for p in range(PAIRS)]
            bqk_sb = w_pool.tile([1, 512], bf16, tag="bqk", name="bqk")
            bv_sb = w_pool.tile([1, J], bf16, tag="bv", name="bv")
            ones_sb = w_pool.tile([1, 512], bf16, tag="ones", name="ones")

            qt_sb = [qk_pool.tile([128, seq], bf16, tag=f"qt{p}",
                                  name=f"qt{p}") for p in range(PAIRS)]
            kt_sb = [qk_pool.tile([128, seq], bf16, tag=f"kt{p}",
                                  name=f"kt{p}") for p in range(PAIRS)]
            v_sb = [v_pool.tile([128, HL, DH + 1], bf16, tag=f"v{s}",
                                name=f"v{s}") for s in range(KB)]
            ot_sb = [ot_pool.tile([128, seq], bf16, tag=f"ot{p}",
                                  name=f"ot{p}") for p in range(PAIRS)]

            for k in range(KT):
                nc.sync.dma_start(out=xt_sb[k], in_=xT_d[k * 128:(k + 1) * 128, :])
                nc.sync.dma_start(out=wq_sb[k], in_=wq_d[k * 128:(k + 1) * 128, :])
                nc.sync.dma_start(out=wk_sb[k], in_=wk_d[k * 128:(k + 1) * 128, :])
                nc.sync.dma_start(out=wv_sb[k], in_=wv_d[k * 128:(k + 1) * 128, :])
            for p in range(PAIRS):
                nc.sync.dma_start(out=wo_sb[p], in_=wo_d[p * 128:(p + 1) * 128, :])
            nc.sync.dma_start(out=bqk_sb, in_=bqk_d)
            nc.sync.dma_start(out=bv_sb, in_=bv_d)
            nc.vector.memset(ones_sb, 1.0)
            for s in range(KB):
                nc.vector.memset(v_sb[s][:, :, DH:DH + 1], 1.0)

            # ---- emission helpers ----
            def emit_qkT_unit(which, p, nck):
                """One [128, NCH] chunk of Q^T or K^T for head-pair p."""
                w_t = wq_sb if which == 0 else wk_sb
                dst = qt_sb[p] if which == 0 else kt_sb[p]
                bias_col = which * 2 + p
                ps = mm_pool.tile([128, 512], f32, tag="mm",
                                  name=f"psqk{which}{p}{nck}")
                for k in range(KT):
                    nc.tensor.matmul(
                        ps[:, :NCH],
                        lhsT=w_t[k][:, p * 128:(p + 1) * 128],
                        rhs=xt_sb[k][:, nck * NCH:(nck + 1) * NCH],
                        start=(k == 0), stop=False)
                nc.tensor.matmul(
                    ps[:, :NCH],
                    lhsT=bqk_sb[0:1, bias_col * 128:(bias_col + 1) * 128],
                    rhs=ones_sb[0:1, :NCH],
                    start=False, stop=True)
                nc.vector.tensor_copy(
                    dst[:, nck * NCH:(nck + 1) * NCH], ps[:, :NCH])

            def emit_v_unit(s):
                """V s-block: [128, J] + bias, stored as [128, HL, DH+1]."""
                ps = mm_pool.tile([128, 512], f32, tag="mm", name=f"psv{s}")
                for k in range(KT):
                    nc.tensor.matmul(
                        ps[:, :J],
                        lhsT=xt_sb[k][:, s * 128:(s + 1) * 128],
                        rhs=wv_sb[k],
                        start=(k == 0), stop=False)
                nc.tensor.matmul(ps[:, :J], lhsT=ones_sb[0:1, :128], rhs=bv_sb,
                                 start=False, stop=True)
                nc.vector.tensor_copy(
                    v_sb[s][:, :, 0:DH],
                    ps[:, :J].rearrange("p (h d) -> p h d", h=HL))

            pt_tiles = {}

            def emit_sT(p, c):
                """Scores^T + exp for head-pair p, query chunk c."""
                pts = []
                for h01 in range(2):
                    pt = pt_pool.tile([128, KB, QC], bf16, tag=f"pt{h01}",
                                      name=f"pt{p}{c}{h01}")
                    pts.append(pt)
                    for g in range(G2):
                        ps = psS_pool.tile([128, 2, QC], f32, tag="psS",
                                           name=f"psS{p}{c}{h01}{g}")
                        for t in range(2):
                            kb = 2 * g + t
                            nc.tensor.matmul(
                                ps[:, t, :],
                                lhsT=kt_sb[p][h01 * 64:(h01 + 1) * 64,
                                              kb * 128:(kb + 1) * 128],
                                rhs=qt_sb[p][h01 * 64:(h01 + 1) * 64,
                                             c * QC:(c + 1) * QC],
                                start=True, stop=True,
                                tile_position=(h01 * 64, 0))
                        nc.scalar.activation(
                            out=pt[:, 2 * g:2 * g + 2, :], in_=ps,
                            func=AF.Exp, scale=0.125)
                pt_tiles[(p, c)] = pts

            def emit_av(p, c):
                """attn @ [V|1], reciprocal, normalize, build O^T pair tile."""
                pts = pt_tiles.pop((p, c))
                for h01 in range(2):
                    h = p * 2 + h01
                    pso = psO_pool.tile([DH + 1, QC], f32, tag="psO",
                                        name=f"psO{p}{c}{h01}")
                    for kb in range(KB):
                        nc.tensor.matmul(
                            pso,
                            lhsT=v_sb[kb][:, h, :],
                            rhs=pts[h01][:, kb, :],
                            start=(kb == 0), stop=(kb == KB - 1))
                    # Softmax denominator: row 64 of pso holds the rowsums.
                    # DVE lanes cannot cross partitions and the native
                    # reciprocal costs 8 cyc/elem/lane, so: copy the [1, QC]
                    # rowsum row to SBUF, bounce it through DRAM reshaped to
                    # [64, QC/64] (64 lanes -> cheap reciprocal), bounce the
                    # reciprocal back to DRAM, and read it in replicated to
                    # [64, QC] with a step-0 DRAM AP.
                    W8 = QC // 64
                    rs1 = sm_pool.tile([DH + 1, QC], f32, tag="rs1",
                                       name=f"rs1{p}{c}{h01}")
                    nc.vector.tensor_copy(rs1[DH:DH + 1, :], pso[DH:DH + 1, :])
                    ds = dram_pool.tile([1, QC], f32, tag="ds",
                                        name=f"ds{p}{c}{h01}")
                    nc.sync.dma_start(out=ds, in_=rs1[DH:DH + 1, :])
                    dsap = ds[0:1, :]
                    rs64 = sm_pool.tile([64, W8], f32, tag="rs64",
                                        name=f"rs64{p}{c}{h01}")
                    nc.sync.dma_start(
                        out=rs64,
                        in_=bass.AP(tensor=dsap.tensor, offset=dsap.offset,
                                    ap=[[W8, 64], [1, W8]]))
                    rr64 = sm_pool.tile([64, W8], f32, tag="rr64",
                                        name=f"rr64{p}{c}{h01}")
                    nc.vector.reciprocal(out=rr64, in_=rs64)
                    ds2 = dram_pool.tile([1, QC], f32, tag="ds2",
                                         name=f"ds2{p}{c}{h01}")
                    ds2ap = ds2[0:1, :]
                    nc.sync.dma_start(
                        out=bass.AP(tensor=ds2ap.tensor, offset=ds2ap.offset,
                                    ap=[[W8, 64], [1, W8]]),
                        in_=rr64)
                    rb = sm_pool.tile([64, QC], f32, tag="rb",
                                      name=f"rb{p}{c}{h01}")
                    nc.sync.dma_start(
                        out=rb,
                        in_=bass.AP(tensor=ds2ap.tensor, offset=ds2ap.offset,
                                    ap=[[0, 64], [1, QC]]))
                    if h01 == 0:
                        nc.vector.tensor_mul(
                            ot_sb[p][0:64, c * QC:(c + 1) * QC],
                            pso[0:DH, :], rb)
                    else:
                        tmp = sm_pool.tile([64, QC], bf16, tag="ottmp",
                                           name=f"ottmp{p}{c}")
                        nc.vector.tensor_mul(tmp, pso[0:DH, :], rb)
                        nc.sync.dma_start(
                            out=ot_sb[p][64:128, c * QC:(c + 1) * QC],
                            in_=tmp)

            def emit_y(qb):
                """Output-projection partial for query block qb."""
                yb = yb_pool.tile([128, D], bf16, tag="yb", name=f"yb{qb}")
                for nchunk in range(D // 512):
                    ps = mm_pool.tile([128, 512], f32, tag="mm",
                                      name=f"psy{qb}{nchunk}")
                    for p in range(PAIRS):
                        nc.tensor.matmul(
                            ps,
                            lhsT=ot_sb[p][:, qb * 128:(qb + 1) * 128],
                            rhs=wo_sb[p][:, nchunk * 512:(nchunk + 1) * 512],
                            start=(p == 0), stop=(p == PAIRS - 1))
                    nc.vector.tensor_copy(yb[:, nchunk * 512:(nchunk + 1) * 512], ps)
                nc.sync.dma_start(out=y_d[qb * 128:(qb + 1) * 128, :], in_=yb)

            # ---- emission schedule ----
            for nck in range(NP):
                emit_qkT_unit(0, 0, nck)   # Q^T pair 0
            for nck in range(NP):
                emit_qkT_unit(1, 0, nck)   # K^T pair 0
            emit_sT(0, 0)
            for s in range(KB):
                emit_v_unit(s)

            fillers = [(0, 1, nck) for nck in range(NP)] + \
                      [(1, 1, nck) for nck in range(NP)]
            steps = [(p, c) for p in range(PAIRS) for c in range(NQ)]
            prev = steps[0]
            for (p, c) in steps[1:]:
                if p == 1:   # pair-1 Q^T/K^T must precede pair-1 scores
                    while fillers:
                        emit_qkT_unit(*fillers.pop(0))
                emit_sT(p, c)
                emit_av(*prev)
                prev = (p, c)
                if p == 0 and fillers:
                    npop = -(-len(fillers) // max(1, NQ - 1 - c))
                    for _ in range(npop):
                        if fillers:
                            emit_qkT_unit(*fillers.pop(0))
            while fillers:
                emit_qkT_unit(*fillers.pop(0))
            emit_av(*prev)
            for qb in range(seq // 128):
                emit_y(qb)

    nc.compile()
    return nc


def _get_module(seq=S):
    if seq not in _cache:
        _cache[seq] = _build_module(seq)
    return _cache[seq]


def _make_in_maps(x, Wq, bq, Wk, bk, Wv, bv, Wo):
    import ml_dtypes
    bf16 = ml_dtypes.bfloat16
    in_maps = []
    for c in range(NCORES):
        b, hg = divmod(c, 4)
        js = slice(hg * J, (hg + 1) * J)
        bqk = np.concatenate(
            [np.asarray(bq[js], np.float32),
             np.asarray(bk[js], np.float32)]).reshape(1, 512).astype(bf16)
        in_maps.append({
            "xT": np.ascontiguousarray(np.asarray(x[b], np.float32).T).astype(bf16),
            "wq": np.ascontiguousarray(np.asarray(Wq, np.float32)[:, js]).astype(bf16),
            "wk": np.ascontiguousarray(np.asarray(Wk, np.float32)[:, js]).astype(bf16),
            "wv": np.ascontiguousarray(np.asarray(Wv, np.float32)[:, js]).astype(bf16),
            "wo": np.ascontiguousarray(np.asarray(Wo, np.float32)[js, :]).astype(bf16),
            "bqk": np.ascontiguousarray(bqk),
            "bv": np.asarray(bv[js], np.float32).reshape(1, J).astype(bf16),
        })
    return in_maps


def _gather(results, bo):
    y = np.zeros((B, S, D), np.float32)
    for b in range(B):
        acc = np.zeros((S, D), np.float32)
        for hg in range(4):
            acc += np.asarray(results[b * 4 + hg]["y"], np.float32)
        y[b] = acc + np.asarray(bo, np.float32)[None, :]
    return y


def run_on_hw(inputs, trace=False, **kwargs):
    """Returns (y_full, BassKernelResults)."""
    from concourse.bass_utils import run_bass_kernel_spmd
    nc = _get_module()
    in_maps = _make_in_maps(
        inputs["x"], inputs["Wq"], inputs["bq"], inputs["Wk"], inputs["bk"],
        inputs["Wv"], inputs["bv"], inputs["Wo"])
    res = run_bass_kernel_spmd(nc, in_maps, core_ids=list(range(NCORES)),
                               trace=trace, **kwargs)
    y = _gather(res.results, inputs["bo"])
    return y, res


def kernel(x, Wq, bq, Wk, bk, Wv, bv, Wo, bo):
    y, _ = run_on_hw(dict(x=x, Wq=Wq, bq=bq, Wk=Wk, bk=bk, Wv=Wv, bv=bv,
                          Wo=Wo, bo=bo))
    return y
